# revision 1
# baseline (speedup 1.0000x reference)
"""ConvLattice (permutohedral lattice conv / GNN message passing) on 8 TRN2 cores.

out[i] = concat_k(lattice[nbr[i,k]]) @ W + b   for i in [0, N)

Strategy: shard vertices across the 8 cores, replicate lattice_values/weight/
bias. Each core gathers its 125k x 9 neighbor rows (128 B each) from its HBM
copy of the lattice with SWDGE indirect DMAs (the TRN2 indirect DMA consumes
one offset per destination partition, so each instruction fetches 128 rows;
measured issue rate ~1.42us/instruction, engine-serial, which is the kernel's
floor), block-transposes them on the vector engine into
contraction-on-partitions layout, and accumulates the K*D=288-deep GEMM on
the tensor engine in 3 chunks (128+128+32).

The neighbor-index tensor is pre-permuted on the host so that:
  gather      -> S[32*k'+j0, 32*m+d]  = lat[nbr[base+32*m+j0, 4*q+k'], d]
  DVE 32x32 T -> R[32*k'+d, 32*m+j0]  = feature d of nbr(4q+k') of vertex
so R[:, 512*s:512*(s+1)] is directly the moving operand of a [C=128, N=512]
matmul. Outputs are written filter-major ([32, NOUT] per core) and
transposed back to [N, 32] on the host.

The last 72 vertices of each core's 125k shard are processed by a dedicated
128-vertex tail block (9 gather instructions instead of a full tile's 144).
"""

import numpy as np

N = 1_000_000
D = 32
K = 9
F = 32
NCORES = 8
NS = N // NCORES          # vertices per core
VSUP = 2048               # vertices per super-tile
SUB = 512                 # vertices per matmul (PSUM free dim)
TF = NS // VSUP           # full super-tiles (61)
TAILV = NS - TF * VSUP    # remaining vertices (72)
TAILP = 128 if TAILV else 0   # tail padded to one 128-vertex block
NOUT = TF * VSUP + TAILP  # output columns per core
IDXC = 144                # idx columns per full super-tile: 64 + 64 + 16
IDXT = 9 if TAILV else 0  # idx columns for the tail block: 4 + 4 + 1
NQUEUES = 4
IDX_SPLIT = 4             # upload idx in this many column-range DMAs

_COMPILED = {}


def _indirect_gather(gps, out, in_, idx_ap, queue_name):
    """Per-partition indirect row gather (same lowering as bass's
    indirect_dma_start for the gather direction) with a selectable SWDGE
    queue so in-flight DMAs spread across the queue rings."""
    import concourse.mybir as mybir

    out_ap = gps.lower_ap_dma(out, for_indirect_dma=True)
    in_ap = gps.lower_ap_dma(in_, for_indirect_dma=True)
    assert len(in_ap) == 1 and len(out_ap) == 1
    offset_ap = gps.lower_ap_dma(idx_ap)
    assert len(offset_ap) == 1
    in_ap.append(offset_ap[0])
    ap_shape = in_.shape
    coef = 1
    for i in range(1, len(ap_shape)):
        coef *= ap_shape[i]
    in_ap[0].dynamic_ap_info = mybir.DynamicAccessPatternInfo(
        c=0,
        actual_ap=out.ap,
        indirect_dim_max_index=ap_shape[0],
        offset_expr=[
            mybir.DynamicAccessPatternOffsetExpr(
                coef=coef,
                aff_expr=mybir.DynamicAccessPatternOffsetExprAffExpr(
                    kind="IndirectArgId", arg_id=1
                ),
            )
        ],
    )
    return gps.add_instruction(
        mybir.InstDMACopy(
            name=gps.bass.get_next_instruction_name(),
            queue=queue_name,
            mode="Copy",
            ins=in_ap,
            outs=out_ap,
            oob_is_err=True,
            cce_op=mybir.AluOpType.bypass,
        )
    )


def _build_nc(n_lat, n_full_tiles, tailp, mm_dtype_name="float32"):
    import concourse.bacc as bacc
    import concourse.mybir as mybir
    import concourse.tile as tile

    f32 = mybir.dt.float32
    i32 = mybir.dt.int32
    mm_dt = getattr(mybir.dt, mm_dtype_name)

    n_idx_cols = n_full_tiles * IDXC + (IDXT if tailp else 0)
    n_out = n_full_tiles * VSUP + tailp

    nc = bacc.Bacc(
        "TRN2",
        target_bir_lowering=False,
        debug=False,
        enable_asserts=False,
        num_devices=NCORES,
        num_swdge_queues=NQUEUES,
    )
    lat = nc.dram_tensor("lat", [n_lat, D], f32, kind="ExternalInput").ap()
    idx = nc.dram_tensor("idx", [128, n_idx_cols], i32, kind="ExternalInput").ap()
    w = nc.dram_tensor("w", [128, 96], f32, kind="ExternalInput").ap()
    b = nc.dram_tensor("b", [128, 1], f32, kind="ExternalInput").ap()
    out = nc.dram_tensor("out", [F, n_out], f32, kind="ExternalOutput").ap()

    def qname(j):
        q = j % NQUEUES
        return f"qPoolDynamic{q or ''}"

    with tile.TileContext(nc) as tc:
        with (
            tc.tile_pool(name="const", bufs=1) as cpool,
            tc.tile_pool(name="gather", bufs=3) as gpool,
            tc.tile_pool(name="trans", bufs=3) as tpool,
            tc.tile_pool(name="outp", bufs=3) as opool,
            tc.tile_pool(name="psum", bufs=2, space="PSUM") as ppool,
        ):
            idx_sb = cpool.tile([128, n_idx_cols], i32)
            w_sb = cpool.tile([128, 96], mm_dt)
            b_sb = cpool.tile([128, 1], f32)
            # Split the idx upload so the first gathers don't wait on the
            # whole 4.6 MB transfer.
            step = (n_idx_cols + IDX_SPLIT - 1) // IDX_SPLIT
            for s0c in range(0, n_idx_cols, step):
                s1c = min(s0c + step, n_idx_cols)
                nc.sync.dma_start(out=idx_sb[:, s0c:s1c], in_=idx[:, s0c:s1c])
            if mm_dt == f32:
                nc.sync.dma_start(out=w_sb[:], in_=w[:, :])
            else:
                nc.gpsimd.dma_start(out=w_sb[:], in_=w[:, :])
            nc.sync.dma_start(out=b_sb[:], in_=b[:, :])

            for t in range(n_full_tiles):
                c0 = t * IDXC
                s0 = gpool.tile([128, VSUP], mm_dt, tag="s0")
                s1 = gpool.tile([128, VSUP], mm_dt, tag="s1")
                s2 = gpool.tile([128, SUB], mm_dt, tag="s2")
                # HW indirect DMA consumes exactly one offset per destination
                # partition row, so each instruction gathers 128 rows. Spread
                # instructions round-robin over the SWDGE queues.
                for m in range(64):
                    _indirect_gather(
                        nc.gpsimd, s0[:, 32 * m:32 * m + 32], lat[:, :],
                        idx_sb[:, c0 + m:c0 + m + 1], qname(m))
                for m in range(64):
                    _indirect_gather(
                        nc.gpsimd, s1[:, 32 * m:32 * m + 32], lat[:, :],
                        idx_sb[:, c0 + 64 + m:c0 + 64 + m + 1], qname(m))
                for m in range(16):
                    _indirect_gather(
                        nc.gpsimd, s2[:, 32 * m:32 * m + 32], lat[:, :],
                        idx_sb[:, c0 + 128 + m:c0 + 128 + m + 1], qname(m))
                r0 = tpool.tile([128, VSUP], mm_dt, tag="r0")
                r1 = tpool.tile([128, VSUP], mm_dt, tag="r1")
                r2p = tpool.tile([128, SUB], mm_dt, tag="r2p")
                r2 = tpool.tile([32, VSUP], mm_dt, tag="r2")
                nc.vector.transpose(out=r0[:], in_=s0[:])
                nc.vector.transpose(out=r1[:], in_=s1[:])
                nc.vector.transpose(out=r2p[:], in_=s2[:])
                # r2p[32s+d, 32m2+j0] holds sub-tile s; move each 32-partition
                # group down to partitions 0:32 so matmul operands stay at
                # partition base 0 (cross-quadrant 32-partition DVE copy).
                for s in range(4):
                    nc.vector.tensor_copy(
                        out=r2[0:32, SUB * s:SUB * (s + 1)],
                        in_=r2p[32 * s:32 * s + 32, :],
                    )

                base = t * VSUP
                for s in range(4):
                    ps = ppool.tile([32, SUB], f32, tag=f"ps{s}")
                    nc.tensor.matmul(
                        out=ps[:],
                        lhsT=w_sb[:, 0:32],
                        rhs=r0[:, SUB * s:SUB * (s + 1)],
                        start=True,
                        stop=False,
                    )
                    nc.tensor.matmul(
                        out=ps[:],
                        lhsT=w_sb[:, 32:64],
                        rhs=r1[:, SUB * s:SUB * (s + 1)],
                        start=False,
                        stop=False,
                    )
                    nc.tensor.matmul(
                        out=ps[:],
                        lhsT=w_sb[0:32, 64:96],
                        rhs=r2[0:32, SUB * s:SUB * (s + 1)],
                        start=False,
                        stop=True,
                    )
                    ob = opool.tile([32, SUB], f32, tag=f"ob{s}")
                    nc.vector.tensor_tensor(
                        out=ob[:],
                        in0=ps[:],
                        in1=b_sb[0:32, 0:1].to_broadcast([32, SUB]),
                        op=mybir.AluOpType.add,
                    )
                    nc.sync.dma_start(
                        out=out[:, base + SUB * s:base + SUB * (s + 1)],
                        in_=ob[:],
                    )

            if tailp:
                # 128-vertex tail block: vertex v = TF*VSUP + 32*a + j0.
                # s0t[32k'+j0, 32m+d] = lat[nbr[v(m,j0), k'], d]   (k'=0..3)
                # s1t likewise for k'=4..7; s2t[32a+j0, d] for k=8.
                c0 = n_full_tiles * IDXC
                s0t = gpool.tile([128, 128], mm_dt, tag="s0t")
                s1t = gpool.tile([128, 128], mm_dt, tag="s1t")
                s2t = gpool.tile([128, 32], mm_dt, tag="s2t")
                for m in range(4):
                    _indirect_gather(
                        nc.gpsimd, s0t[:, 32 * m:32 * m + 32], lat[:, :],
                        idx_sb[:, c0 + m:c0 + m + 1], qname(m))
                for m in range(4):
                    _indirect_gather(
                        nc.gpsimd, s1t[:, 32 * m:32 * m + 32], lat[:, :],
                        idx_sb[:, c0 + 4 + m:c0 + 4 + m + 1], qname(m))
                _indirect_gather(
                    nc.gpsimd, s2t[:, 0:32], lat[:, :],
                    idx_sb[:, c0 + 8:c0 + 9], qname(0))
                r0t = tpool.tile([128, 128], mm_dt, tag="r0t")
                r1t = tpool.tile([128, 128], mm_dt, tag="r1t")
                r2pt = tpool.tile([128, 32], mm_dt, tag="r2pt")
                r2t = tpool.tile([32, 128], mm_dt, tag="r2t")
                nc.vector.transpose(out=r0t[:], in_=s0t[:])
                nc.vector.transpose(out=r1t[:], in_=s1t[:])
                nc.vector.transpose(out=r2pt[:], in_=s2t[:])
                for a in range(4):
                    nc.vector.tensor_copy(
                        out=r2t[0:32, 32 * a:32 * a + 32],
                        in_=r2pt[32 * a:32 * a + 32, :],
                    )
                ps = ppool.tile([32, SUB], f32, tag="ps0")
                nc.tensor.matmul(out=ps[:, 0:128], lhsT=w_sb[:, 0:32],
                                 rhs=r0t[:, :], start=True, stop=False)
                nc.tensor.matmul(out=ps[:, 0:128], lhsT=w_sb[:, 32:64],
                                 rhs=r1t[:, :], start=False, stop=False)
                nc.tensor.matmul(out=ps[:, 0:128], lhsT=w_sb[0:32, 64:96],
                                 rhs=r2t[0:32, :], start=False, stop=True)
                ob = opool.tile([32, SUB], f32, tag="ob0")
                nc.vector.tensor_tensor(
                    out=ob[:, 0:128], in0=ps[:, 0:128],
                    in1=b_sb[0:32, 0:1].to_broadcast([32, 128]),
                    op=mybir.AluOpType.add)
                nc.sync.dma_start(
                    out=out[:, n_full_tiles * VSUP:n_full_tiles * VSUP + 128],
                    in_=ob[:, 0:128])
    nc.compile()
    return nc


def get_nc(n_lat=N, n_full_tiles=TF, tailp=TAILP, mm_dtype_name="float32"):
    key = (n_lat, n_full_tiles, tailp, mm_dtype_name)
    if key not in _COMPILED:
        _COMPILED[key] = _build_nc(n_lat, n_full_tiles, tailp, mm_dtype_name)
    return _COMPILED[key]


def prep_idx(nbr, n_full_tiles=TF, tailp=TAILP):
    """Permute a [ns, 9] int32 neighbor-index shard into the gather layout.

    Returns [128, n_full_tiles*IDXC + (9 if tail else 0)] int32:
      per super-tile t, cols [0:64)  = I0[32*k'+j0, m] = nbr[2048t+32m+j0, k']
                    cols [64:128)    = same for k' in 4..8
                    cols [128:144)   = I2[32*s+j0, m2] = nbr[2048t+512s+32m2+j0, 8]
      tail block (vertices TF*2048 .. ns, zero-padded to 128):
        cols [0:4) = nbr[base+32m+j0, k'] k'=0..3; [4:8) k'=4..7; [8] = k=8.
    """
    ns = nbr.shape[0]
    nfull = n_full_tiles * VSUP
    a = np.asarray(nbr[:nfull], np.int32)
    A = a.reshape(n_full_tiles, 64, 32, K)             # [t, m, j0, k]
    i0 = A[..., 0:4].transpose(0, 3, 2, 1).reshape(n_full_tiles, 128, 64)
    i1 = A[..., 4:8].transpose(0, 3, 2, 1).reshape(n_full_tiles, 128, 64)
    # I2[32s+j0, m2] = nbr[2048t + 512s + 32m2 + j0, 8]
    i2 = (
        a[:, 8].reshape(n_full_tiles, 4, 16, 32)       # [t, s, m2, j0]
        .transpose(0, 1, 3, 2)                         # [t, s, j0, m2]
        .reshape(n_full_tiles, 128, 16)
    )
    idx = np.concatenate([i0, i1, i2], axis=2)         # [t, 128, 144]
    full = idx.transpose(1, 0, 2).reshape(128, n_full_tiles * IDXC)
    if not tailp:
        return np.ascontiguousarray(full)
    at = np.zeros((tailp, K), np.int32)
    at[:ns - nfull] = nbr[nfull:]
    At = at.reshape(4, 32, K)                          # [m, j0, k]
    t0 = At[..., 0:4].transpose(2, 1, 0).reshape(128, 4)
    t1 = At[..., 4:8].transpose(2, 1, 0).reshape(128, 4)
    t2 = at[:, 8].reshape(128, 1)
    return np.ascontiguousarray(np.concatenate([full, t0, t1, t2], axis=1))


def pack_weights(weight, bias_param):
    wp = np.zeros((128, 96), np.float32)
    wp[:, 0:32] = weight[0:128]
    wp[:, 32:64] = weight[128:256]
    wp[0:32, 64:96] = weight[256:288]
    bp = np.ascontiguousarray(np.tile(np.asarray(bias_param, np.float32), 4)[:, None])
    return wp, bp


def make_in_maps(lattice_values, neighbor_indices, weight, bias_param):
    lat = np.ascontiguousarray(np.asarray(lattice_values, np.float32))
    nbr = np.asarray(neighbor_indices, np.int32)
    wp, bp = pack_weights(np.asarray(weight, np.float32), bias_param)
    return [
        {
            "lat": lat,
            "idx": prep_idx(nbr[c * NS:(c + 1) * NS]),
            "w": wp,
            "b": bp,
        }
        for c in range(NCORES)
    ]


def kernel(lattice_values, neighbor_indices, weight, bias_param):
    from concourse import bass_utils

    nc = get_nc()
    in_maps = make_in_maps(lattice_values, neighbor_indices, weight, bias_param)
    res = bass_utils.run_bass_kernel_spmd(nc, in_maps, core_ids=list(range(NCORES)))
    return np.ascontiguousarray(
        np.concatenate([r["out"][:, :NS].T for r in res.results], axis=0)
    ).astype(np.float32)



# revision 2
# speedup vs baseline: 1.0073x; 1.0073x over previous
"""ConvLattice (permutohedral lattice conv / GNN message passing) on 8 TRN2 cores.

out[i] = concat_k(lattice[nbr[i,k]]) @ W + b   for i in [0, N)

Strategy: shard vertices across the 8 cores, replicate lattice_values/weight/
bias. Each core gathers its 125k x 9 neighbor rows from its HBM copy of the
lattice with SWDGE indirect DMAs (one offset per destination partition -> each
instruction fetches 128 rows; the 16 SDMA engines each process 8 rows per
instruction as one packet). The lattice is converted to bf16 on the host so a
gathered row is 64 B (512 B/engine-packet instead of 1 KB), roughly halving
the per-packet data time; the per-packet fixed cost (~50 ns) dominates either
way, so the gather runs at ~0.55-0.7 ns/row.

Gathered tiles are block-transposed on the vector engine into
contraction-on-partitions layout and the K*D=288-deep GEMM accumulates on the
tensor engine in 3 chunks (128+128+32), all in bf16 with f32 PSUM.

The neighbor-index tensor is pre-permuted on the host so that:
  gather      -> S[32*k'+j0, 32*m+d]  = lat[nbr[base+32*m+j0, 4*q+k'], d]
  DVE 32x32 T -> R[32*k'+d, 32*m+j0]  = feature d of nbr(4q+k') of vertex
so R[:, 512*s:512*(s+1)] is directly the moving operand of a [C=128, N=512]
matmul. Outputs are written filter-major ([32, NOUT] bf16 per core, one DMA
per 2048-vertex super-tile) and transposed back to [N, 32] f32 on the host.

The last 72 vertices of each core's 125k shard are processed by a dedicated
128-vertex tail block (9 gather instructions instead of a full tile's 144).
"""

import numpy as np

N = 1_000_000
D = 32
K = 9
F = 32
NCORES = 8
NS = N // NCORES          # vertices per core
VSUP = 2048               # vertices per super-tile
SUB = 512                 # vertices per matmul (PSUM free dim)
TF = NS // VSUP           # full super-tiles (61)
TAILV = NS - TF * VSUP    # remaining vertices (72)
TAILP = 128 if TAILV else 0   # tail padded to one 128-vertex block
NOUT = TF * VSUP + TAILP  # output columns per core
IDXC = 144                # idx columns per full super-tile: 64 + 64 + 16
IDXT = 9 if TAILV else 0  # idx columns for the tail block: 4 + 4 + 1
NQUEUES = 4
IDX_SPLIT = 4             # upload idx in this many column-range DMAs

_COMPILED = {}


def _indirect_gather(gps, out, in_, idx_ap, queue_name):
    """Per-partition indirect row gather (same lowering as bass's
    indirect_dma_start for the gather direction) with a selectable SWDGE
    queue so in-flight DMAs spread across the queue rings."""
    import concourse.mybir as mybir

    out_ap = gps.lower_ap_dma(out, for_indirect_dma=True)
    in_ap = gps.lower_ap_dma(in_, for_indirect_dma=True)
    assert len(in_ap) == 1 and len(out_ap) == 1
    offset_ap = gps.lower_ap_dma(idx_ap)
    assert len(offset_ap) == 1
    in_ap.append(offset_ap[0])
    ap_shape = in_.shape
    coef = 1
    for i in range(1, len(ap_shape)):
        coef *= ap_shape[i]
    in_ap[0].dynamic_ap_info = mybir.DynamicAccessPatternInfo(
        c=0,
        actual_ap=out.ap,
        indirect_dim_max_index=ap_shape[0],
        offset_expr=[
            mybir.DynamicAccessPatternOffsetExpr(
                coef=coef,
                aff_expr=mybir.DynamicAccessPatternOffsetExprAffExpr(
                    kind="IndirectArgId", arg_id=1
                ),
            )
        ],
    )
    return gps.add_instruction(
        mybir.InstDMACopy(
            name=gps.bass.get_next_instruction_name(),
            queue=queue_name,
            mode="Copy",
            ins=in_ap,
            outs=out_ap,
            oob_is_err=True,
            cce_op=mybir.AluOpType.bypass,
        )
    )


def _build_nc(n_lat, n_full_tiles, tailp):
    import concourse.bacc as bacc
    import concourse.mybir as mybir
    import concourse.tile as tile

    f32 = mybir.dt.float32
    bf16 = mybir.dt.bfloat16
    i32 = mybir.dt.int32

    n_idx_cols = n_full_tiles * IDXC + (IDXT if tailp else 0)
    n_out = n_full_tiles * VSUP + tailp

    nc = bacc.Bacc(
        "TRN2",
        target_bir_lowering=False,
        debug=False,
        enable_asserts=False,
        num_devices=NCORES,
        num_swdge_queues=NQUEUES,
    )
    lat = nc.dram_tensor("lat", [n_lat, D], bf16, kind="ExternalInput").ap()
    idx = nc.dram_tensor("idx", [128, n_idx_cols], i32, kind="ExternalInput").ap()
    w = nc.dram_tensor("w", [128, 96], bf16, kind="ExternalInput").ap()
    b = nc.dram_tensor("b", [128, 1], f32, kind="ExternalInput").ap()
    out = nc.dram_tensor("out", [F, n_out], bf16, kind="ExternalOutput").ap()

    def qname(j):
        q = j % NQUEUES
        return f"qPoolDynamic{q or ''}"

    with tile.TileContext(nc) as tc:
        with (
            tc.tile_pool(name="const", bufs=1) as cpool,
            tc.tile_pool(name="gather", bufs=3) as gpool,
            tc.tile_pool(name="trans", bufs=3) as tpool,
            tc.tile_pool(name="outp", bufs=3) as opool,
            tc.tile_pool(name="psum", bufs=2, space="PSUM") as ppool,
        ):
            idx_sb = cpool.tile([128, n_idx_cols], i32)
            w_sb = cpool.tile([128, 96], bf16)
            b_sb = cpool.tile([128, 1], f32)
            # Split the idx upload so the first gathers don't wait on the
            # whole 4.6 MB transfer.
            step = (n_idx_cols + IDX_SPLIT - 1) // IDX_SPLIT
            for s0c in range(0, n_idx_cols, step):
                s1c = min(s0c + step, n_idx_cols)
                nc.sync.dma_start(out=idx_sb[:, s0c:s1c], in_=idx[:, s0c:s1c])
            nc.sync.dma_start(out=w_sb[:], in_=w[:, :])
            nc.sync.dma_start(out=b_sb[:], in_=b[:, :])

            for t in range(n_full_tiles):
                c0 = t * IDXC
                s0 = gpool.tile([128, VSUP], bf16, tag="s0")
                s1 = gpool.tile([128, VSUP], bf16, tag="s1")
                s2 = gpool.tile([128, SUB], bf16, tag="s2")
                # HW indirect DMA consumes exactly one offset per destination
                # partition row, so each instruction gathers 128 rows. Spread
                # instructions round-robin over the SWDGE queues.
                for m in range(64):
                    _indirect_gather(
                        nc.gpsimd, s0[:, 32 * m:32 * m + 32], lat[:, :],
                        idx_sb[:, c0 + m:c0 + m + 1], qname(m))
                for m in range(64):
                    _indirect_gather(
                        nc.gpsimd, s1[:, 32 * m:32 * m + 32], lat[:, :],
                        idx_sb[:, c0 + 64 + m:c0 + 64 + m + 1], qname(m))
                for m in range(16):
                    _indirect_gather(
                        nc.gpsimd, s2[:, 32 * m:32 * m + 32], lat[:, :],
                        idx_sb[:, c0 + 128 + m:c0 + 128 + m + 1], qname(m))
                r0 = tpool.tile([128, VSUP], bf16, tag="r0")
                r1 = tpool.tile([128, VSUP], bf16, tag="r1")
                r2p = tpool.tile([128, SUB], bf16, tag="r2p")
                r2 = tpool.tile([32, VSUP], bf16, tag="r2")
                nc.vector.transpose(out=r0[:], in_=s0[:])
                nc.vector.transpose(out=r1[:], in_=s1[:])
                nc.vector.transpose(out=r2p[:], in_=s2[:])
                # r2p[32s+d, 32m2+j0] holds sub-tile s; move each 32-partition
                # group down to partitions 0:32 so matmul operands stay at
                # partition base 0 (cross-quadrant 32-partition DVE copy).
                for s in range(4):
                    nc.vector.tensor_copy(
                        out=r2[0:32, SUB * s:SUB * (s + 1)],
                        in_=r2p[32 * s:32 * s + 32, :],
                    )

                base = t * VSUP
                ob = opool.tile([32, VSUP], bf16, tag="ob")
                for s in range(4):
                    ps = ppool.tile([32, SUB], f32, tag=f"ps{s}")
                    nc.tensor.matmul(
                        out=ps[:],
                        lhsT=w_sb[:, 0:32],
                        rhs=r0[:, SUB * s:SUB * (s + 1)],
                        start=True,
                        stop=False,
                    )
                    nc.tensor.matmul(
                        out=ps[:],
                        lhsT=w_sb[:, 32:64],
                        rhs=r1[:, SUB * s:SUB * (s + 1)],
                        start=False,
                        stop=False,
                    )
                    nc.tensor.matmul(
                        out=ps[:],
                        lhsT=w_sb[0:32, 64:96],
                        rhs=r2[0:32, SUB * s:SUB * (s + 1)],
                        start=False,
                        stop=True,
                    )
                    nc.vector.tensor_tensor(
                        out=ob[:, SUB * s:SUB * (s + 1)],
                        in0=ps[:],
                        in1=b_sb[0:32, 0:1].to_broadcast([32, SUB]),
                        op=mybir.AluOpType.add,
                    )
                nc.sync.dma_start(
                    out=out[:, base:base + VSUP],
                    in_=ob[:],
                )

            if tailp:
                # 128-vertex tail block: vertex v = TF*VSUP + 32*a + j0.
                # s0t[32k'+j0, 32m+d] = lat[nbr[v(m,j0), k'], d]   (k'=0..3)
                # s1t likewise for k'=4..7; s2t[32a+j0, d] for k=8.
                c0 = n_full_tiles * IDXC
                s0t = gpool.tile([128, 128], bf16, tag="s0t")
                s1t = gpool.tile([128, 128], bf16, tag="s1t")
                s2t = gpool.tile([128, 32], bf16, tag="s2t")
                for m in range(4):
                    _indirect_gather(
                        nc.gpsimd, s0t[:, 32 * m:32 * m + 32], lat[:, :],
                        idx_sb[:, c0 + m:c0 + m + 1], qname(m))
                for m in range(4):
                    _indirect_gather(
                        nc.gpsimd, s1t[:, 32 * m:32 * m + 32], lat[:, :],
                        idx_sb[:, c0 + 4 + m:c0 + 4 + m + 1], qname(m))
                _indirect_gather(
                    nc.gpsimd, s2t[:, 0:32], lat[:, :],
                    idx_sb[:, c0 + 8:c0 + 9], qname(0))
                r0t = tpool.tile([128, 128], bf16, tag="r0t")
                r1t = tpool.tile([128, 128], bf16, tag="r1t")
                r2pt = tpool.tile([128, 32], bf16, tag="r2pt")
                r2t = tpool.tile([32, 128], bf16, tag="r2t")
                nc.vector.transpose(out=r0t[:], in_=s0t[:])
                nc.vector.transpose(out=r1t[:], in_=s1t[:])
                nc.vector.transpose(out=r2pt[:], in_=s2t[:])
                for a in range(4):
                    nc.vector.tensor_copy(
                        out=r2t[0:32, 32 * a:32 * a + 32],
                        in_=r2pt[32 * a:32 * a + 32, :],
                    )
                ps = ppool.tile([32, SUB], f32, tag="ps0")
                nc.tensor.matmul(out=ps[:, 0:128], lhsT=w_sb[:, 0:32],
                                 rhs=r0t[:, :], start=True, stop=False)
                nc.tensor.matmul(out=ps[:, 0:128], lhsT=w_sb[:, 32:64],
                                 rhs=r1t[:, :], start=False, stop=False)
                nc.tensor.matmul(out=ps[:, 0:128], lhsT=w_sb[0:32, 64:96],
                                 rhs=r2t[0:32, :], start=False, stop=True)
                ob = opool.tile([32, 128], bf16, tag="obt")
                nc.vector.tensor_tensor(
                    out=ob[:, 0:128], in0=ps[:, 0:128],
                    in1=b_sb[0:32, 0:1].to_broadcast([32, 128]),
                    op=mybir.AluOpType.add)
                nc.sync.dma_start(
                    out=out[:, n_full_tiles * VSUP:n_full_tiles * VSUP + 128],
                    in_=ob[:, 0:128])
    nc.compile()
    return nc


def get_nc(n_lat=N, n_full_tiles=TF, tailp=TAILP):
    key = (n_lat, n_full_tiles, tailp)
    if key not in _COMPILED:
        _COMPILED[key] = _build_nc(n_lat, n_full_tiles, tailp)
    return _COMPILED[key]


def prep_idx(nbr, n_full_tiles=TF, tailp=TAILP):
    """Permute a [ns, 9] int32 neighbor-index shard into the gather layout.

    Returns [128, n_full_tiles*IDXC + (9 if tail else 0)] int32:
      per super-tile t, cols [0:64)  = I0[32*k'+j0, m] = nbr[2048t+32m+j0, k']
                    cols [64:128)    = same for k' in 4..8
                    cols [128:144)   = I2[32*s+j0, m2] = nbr[2048t+512s+32m2+j0, 8]
      tail block (vertices TF*2048 .. ns, zero-padded to 128):
        cols [0:4) = nbr[base+32m+j0, k'] k'=0..3; [4:8) k'=4..7; [8] = k=8.
    """
    ns = nbr.shape[0]
    nfull = n_full_tiles * VSUP
    a = np.asarray(nbr[:nfull], np.int32)
    A = a.reshape(n_full_tiles, 64, 32, K)             # [t, m, j0, k]
    i0 = A[..., 0:4].transpose(0, 3, 2, 1).reshape(n_full_tiles, 128, 64)
    i1 = A[..., 4:8].transpose(0, 3, 2, 1).reshape(n_full_tiles, 128, 64)
    # I2[32s+j0, m2] = nbr[2048t + 512s + 32m2 + j0, 8]
    i2 = (
        a[:, 8].reshape(n_full_tiles, 4, 16, 32)       # [t, s, m2, j0]
        .transpose(0, 1, 3, 2)                         # [t, s, j0, m2]
        .reshape(n_full_tiles, 128, 16)
    )
    idx = np.concatenate([i0, i1, i2], axis=2)         # [t, 128, 144]
    full = idx.transpose(1, 0, 2).reshape(128, n_full_tiles * IDXC)
    if not tailp:
        return np.ascontiguousarray(full)
    at = np.zeros((tailp, K), np.int32)
    at[:ns - nfull] = nbr[nfull:]
    At = at.reshape(4, 32, K)                          # [m, j0, k]
    t0 = At[..., 0:4].transpose(2, 1, 0).reshape(128, 4)
    t1 = At[..., 4:8].transpose(2, 1, 0).reshape(128, 4)
    t2 = at[:, 8].reshape(128, 1)
    return np.ascontiguousarray(np.concatenate([full, t0, t1, t2], axis=1))


def pack_weights(weight, bias_param):
    import ml_dtypes

    wp = np.zeros((128, 96), np.float32)
    wp[:, 0:32] = weight[0:128]
    wp[:, 32:64] = weight[128:256]
    wp[0:32, 64:96] = weight[256:288]
    bp = np.ascontiguousarray(np.tile(np.asarray(bias_param, np.float32), 4)[:, None])
    return wp.astype(ml_dtypes.bfloat16), bp


def make_in_maps(lattice_values, neighbor_indices, weight, bias_param):
    import ml_dtypes

    lat = np.ascontiguousarray(
        np.asarray(lattice_values, np.float32).astype(ml_dtypes.bfloat16)
    )
    nbr = np.asarray(neighbor_indices, np.int32)
    wp, bp = pack_weights(np.asarray(weight, np.float32), bias_param)
    return [
        {
            "lat": lat,
            "idx": prep_idx(nbr[c * NS:(c + 1) * NS]),
            "w": wp,
            "b": bp,
        }
        for c in range(NCORES)
    ]


def kernel(lattice_values, neighbor_indices, weight, bias_param):
    from concourse import bass_utils

    nc = get_nc()
    in_maps = make_in_maps(lattice_values, neighbor_indices, weight, bias_param)
    res = bass_utils.run_bass_kernel_spmd(nc, in_maps, core_ids=list(range(NCORES)))
    return np.ascontiguousarray(
        np.concatenate(
            [np.asarray(r["out"][:, :NS]).astype(np.float32).T for r in res.results],
            axis=0,
        )
    )


# revision 4
# speedup vs baseline: 1.0075x; 1.0002x over previous
"""ConvLattice (permutohedral lattice conv / GNN message passing) on 8 TRN2 cores.

out[i] = concat_k(lattice[nbr[i,k]]) @ W + b   for i in [0, N)

Strategy: shard vertices across the 8 cores, replicate lattice_values/weight/
bias. Each core gathers its 125k x 9 neighbor rows from its HBM copy of the
lattice with SWDGE indirect DMAs (one offset per destination partition -> each
instruction fetches 128 rows; the 16 SDMA engines each process 8 rows per
instruction as one packet). The lattice is converted to bf16 on the host so a
gathered row is 64 B (512 B/engine-packet instead of 1 KB), roughly halving
the per-packet data time; the per-packet fixed cost (~50 ns) dominates either
way, so the gather runs at ~0.55-0.7 ns/row.

Gathered tiles are block-transposed on the vector engine into
contraction-on-partitions layout and the K*D=288-deep GEMM accumulates on the
tensor engine in 3 chunks (128+128+32), all in bf16 with f32 PSUM.

The neighbor-index tensor is pre-permuted on the host so that:
  gather      -> S[32*k'+j0, 32*m+d]  = lat[nbr[base+32*m+j0, 4*q+k'], d]
  DVE 32x32 T -> R[32*k'+d, 32*m+j0]  = feature d of nbr(4q+k') of vertex
so R[:, 512*s:512*(s+1)] is directly the moving operand of a [C=128, N=512]
matmul. Outputs are written filter-major ([32, NOUT] bf16 per core, one DMA
per 2048-vertex super-tile) and transposed back to [N, 32] f32 on the host.

The last 72 vertices of each core's 125k shard are processed by a dedicated
128-vertex tail block (9 gather instructions instead of a full tile's 144).
"""

import numpy as np

N = 1_000_000
D = 32
K = 9
F = 32
NCORES = 8
NS = N // NCORES          # vertices per core
VSUP = 2048               # vertices per super-tile
SUB = 512                 # vertices per matmul (PSUM free dim)
TF = NS // VSUP           # full super-tiles (61)
TAILV = NS - TF * VSUP    # remaining vertices (72)
TAILP = 128 if TAILV else 0   # tail padded to one 128-vertex block
NOUT = TF * VSUP + TAILP  # output columns per core
IDXC = 144                # idx columns per full super-tile: 64 + 64 + 16
IDXT = 9 if TAILV else 0  # idx columns for the tail block: 4 + 4 + 1
NQUEUES = 4
IDX_SPLIT = 8             # upload idx in this many column-range DMAs

_COMPILED = {}


def _indirect_gather(gps, out, in_, idx_ap, queue_name):
    """Per-partition indirect row gather (same lowering as bass's
    indirect_dma_start for the gather direction) with a selectable SWDGE
    queue so in-flight DMAs spread across the queue rings."""
    import concourse.mybir as mybir

    out_ap = gps.lower_ap_dma(out, for_indirect_dma=True)
    in_ap = gps.lower_ap_dma(in_, for_indirect_dma=True)
    assert len(in_ap) == 1 and len(out_ap) == 1
    offset_ap = gps.lower_ap_dma(idx_ap)
    assert len(offset_ap) == 1
    in_ap.append(offset_ap[0])
    ap_shape = in_.shape
    coef = 1
    for i in range(1, len(ap_shape)):
        coef *= ap_shape[i]
    in_ap[0].dynamic_ap_info = mybir.DynamicAccessPatternInfo(
        c=0,
        actual_ap=out.ap,
        indirect_dim_max_index=ap_shape[0],
        offset_expr=[
            mybir.DynamicAccessPatternOffsetExpr(
                coef=coef,
                aff_expr=mybir.DynamicAccessPatternOffsetExprAffExpr(
                    kind="IndirectArgId", arg_id=1
                ),
            )
        ],
    )
    return gps.add_instruction(
        mybir.InstDMACopy(
            name=gps.bass.get_next_instruction_name(),
            queue=queue_name,
            mode="Copy",
            ins=in_ap,
            outs=out_ap,
            oob_is_err=True,
            cce_op=mybir.AluOpType.bypass,
        )
    )


def _build_nc(n_lat, n_full_tiles, tailp):
    import concourse.bacc as bacc
    import concourse.mybir as mybir
    import concourse.tile as tile

    f32 = mybir.dt.float32
    bf16 = mybir.dt.bfloat16
    i32 = mybir.dt.int32

    n_idx_cols = n_full_tiles * IDXC + (IDXT if tailp else 0)
    n_out = n_full_tiles * VSUP + tailp

    nc = bacc.Bacc(
        "TRN2",
        target_bir_lowering=False,
        debug=False,
        enable_asserts=False,
        num_devices=NCORES,
        num_swdge_queues=NQUEUES,
    )
    lat = nc.dram_tensor("lat", [n_lat, D], bf16, kind="ExternalInput").ap()
    idx = nc.dram_tensor("idx", [128, n_idx_cols], i32, kind="ExternalInput").ap()
    w = nc.dram_tensor("w", [128, 96], bf16, kind="ExternalInput").ap()
    b = nc.dram_tensor("b", [128, 1], f32, kind="ExternalInput").ap()
    out = nc.dram_tensor("out", [F, n_out], bf16, kind="ExternalOutput").ap()

    def qname(j):
        q = j % NQUEUES
        return f"qPoolDynamic{q or ''}"

    with tile.TileContext(nc) as tc:
        with (
            tc.tile_pool(name="const", bufs=1) as cpool,
            tc.tile_pool(name="gather", bufs=4) as gpool,
            tc.tile_pool(name="trans", bufs=4) as tpool,
            tc.tile_pool(name="outp", bufs=3) as opool,
            tc.tile_pool(name="psum", bufs=2, space="PSUM") as ppool,
        ):
            idx_sb = cpool.tile([128, n_idx_cols], i32)
            w_sb = cpool.tile([128, 96], bf16)
            b_sb = cpool.tile([128, 1], f32)
            # Split the idx upload so the first gathers don't wait on the
            # whole 4.6 MB transfer.
            step = (n_idx_cols + IDX_SPLIT - 1) // IDX_SPLIT
            for s0c in range(0, n_idx_cols, step):
                s1c = min(s0c + step, n_idx_cols)
                nc.sync.dma_start(out=idx_sb[:, s0c:s1c], in_=idx[:, s0c:s1c])
            nc.sync.dma_start(out=w_sb[:], in_=w[:, :])
            nc.sync.dma_start(out=b_sb[:], in_=b[:, :])

            for t in range(n_full_tiles):
                c0 = t * IDXC
                s0 = gpool.tile([128, VSUP], bf16, tag="s0")
                s1 = gpool.tile([128, VSUP], bf16, tag="s1")
                s2 = gpool.tile([128, SUB], bf16, tag="s2")
                # HW indirect DMA consumes exactly one offset per destination
                # partition row, so each instruction gathers 128 rows. Spread
                # instructions round-robin over the SWDGE queues.
                for m in range(64):
                    _indirect_gather(
                        nc.gpsimd, s0[:, 32 * m:32 * m + 32], lat[:, :],
                        idx_sb[:, c0 + m:c0 + m + 1], qname(m))
                for m in range(64):
                    _indirect_gather(
                        nc.gpsimd, s1[:, 32 * m:32 * m + 32], lat[:, :],
                        idx_sb[:, c0 + 64 + m:c0 + 64 + m + 1], qname(m))
                for m in range(16):
                    _indirect_gather(
                        nc.gpsimd, s2[:, 32 * m:32 * m + 32], lat[:, :],
                        idx_sb[:, c0 + 128 + m:c0 + 128 + m + 1], qname(m))
                r0 = tpool.tile([128, VSUP], bf16, tag="r0")
                r1 = tpool.tile([128, VSUP], bf16, tag="r1")
                r2p = tpool.tile([128, SUB], bf16, tag="r2p")
                r2 = tpool.tile([32, VSUP], bf16, tag="r2")
                nc.vector.transpose(out=r0[:], in_=s0[:])
                nc.vector.transpose(out=r1[:], in_=s1[:])
                nc.vector.transpose(out=r2p[:], in_=s2[:])
                # r2p[32s+d, 32m2+j0] holds sub-tile s; move each 32-partition
                # group down to partitions 0:32 so matmul operands stay at
                # partition base 0 (cross-quadrant 32-partition DVE copy).
                for s in range(4):
                    nc.vector.tensor_copy(
                        out=r2[0:32, SUB * s:SUB * (s + 1)],
                        in_=r2p[32 * s:32 * s + 32, :],
                    )

                base = t * VSUP
                ob = opool.tile([32, VSUP], bf16, tag="ob")
                for s in range(4):
                    ps = ppool.tile([32, SUB], f32, tag=f"ps{s}")
                    nc.tensor.matmul(
                        out=ps[:],
                        lhsT=w_sb[:, 0:32],
                        rhs=r0[:, SUB * s:SUB * (s + 1)],
                        start=True,
                        stop=False,
                    )
                    nc.tensor.matmul(
                        out=ps[:],
                        lhsT=w_sb[:, 32:64],
                        rhs=r1[:, SUB * s:SUB * (s + 1)],
                        start=False,
                        stop=False,
                    )
                    nc.tensor.matmul(
                        out=ps[:],
                        lhsT=w_sb[0:32, 64:96],
                        rhs=r2[0:32, SUB * s:SUB * (s + 1)],
                        start=False,
                        stop=True,
                    )
                    nc.vector.tensor_tensor(
                        out=ob[:, SUB * s:SUB * (s + 1)],
                        in0=ps[:],
                        in1=b_sb[0:32, 0:1].to_broadcast([32, SUB]),
                        op=mybir.AluOpType.add,
                    )
                nc.sync.dma_start(
                    out=out[:, base:base + VSUP],
                    in_=ob[:],
                )

            if tailp:
                # 128-vertex tail block: vertex v = TF*VSUP + 32*a + j0.
                # s0t[32k'+j0, 32m+d] = lat[nbr[v(m,j0), k'], d]   (k'=0..3)
                # s1t likewise for k'=4..7; s2t[32a+j0, d] for k=8.
                c0 = n_full_tiles * IDXC
                s0t = gpool.tile([128, 128], bf16, tag="s0t")
                s1t = gpool.tile([128, 128], bf16, tag="s1t")
                s2t = gpool.tile([128, 32], bf16, tag="s2t")
                for m in range(4):
                    _indirect_gather(
                        nc.gpsimd, s0t[:, 32 * m:32 * m + 32], lat[:, :],
                        idx_sb[:, c0 + m:c0 + m + 1], qname(m))
                for m in range(4):
                    _indirect_gather(
                        nc.gpsimd, s1t[:, 32 * m:32 * m + 32], lat[:, :],
                        idx_sb[:, c0 + 4 + m:c0 + 4 + m + 1], qname(m))
                _indirect_gather(
                    nc.gpsimd, s2t[:, 0:32], lat[:, :],
                    idx_sb[:, c0 + 8:c0 + 9], qname(0))
                r0t = tpool.tile([128, 128], bf16, tag="r0t")
                r1t = tpool.tile([128, 128], bf16, tag="r1t")
                r2pt = tpool.tile([128, 32], bf16, tag="r2pt")
                r2t = tpool.tile([32, 128], bf16, tag="r2t")
                nc.vector.transpose(out=r0t[:], in_=s0t[:])
                nc.vector.transpose(out=r1t[:], in_=s1t[:])
                nc.vector.transpose(out=r2pt[:], in_=s2t[:])
                for a in range(4):
                    nc.vector.tensor_copy(
                        out=r2t[0:32, 32 * a:32 * a + 32],
                        in_=r2pt[32 * a:32 * a + 32, :],
                    )
                ps = ppool.tile([32, SUB], f32, tag="ps0")
                nc.tensor.matmul(out=ps[:, 0:128], lhsT=w_sb[:, 0:32],
                                 rhs=r0t[:, :], start=True, stop=False)
                nc.tensor.matmul(out=ps[:, 0:128], lhsT=w_sb[:, 32:64],
                                 rhs=r1t[:, :], start=False, stop=False)
                nc.tensor.matmul(out=ps[:, 0:128], lhsT=w_sb[0:32, 64:96],
                                 rhs=r2t[0:32, :], start=False, stop=True)
                ob = opool.tile([32, 128], bf16, tag="obt")
                nc.vector.tensor_tensor(
                    out=ob[:, 0:128], in0=ps[:, 0:128],
                    in1=b_sb[0:32, 0:1].to_broadcast([32, 128]),
                    op=mybir.AluOpType.add)
                nc.sync.dma_start(
                    out=out[:, n_full_tiles * VSUP:n_full_tiles * VSUP + 128],
                    in_=ob[:, 0:128])
    nc.compile()
    return nc


def get_nc(n_lat=N, n_full_tiles=TF, tailp=TAILP):
    key = (n_lat, n_full_tiles, tailp)
    if key not in _COMPILED:
        _COMPILED[key] = _build_nc(n_lat, n_full_tiles, tailp)
    return _COMPILED[key]


def prep_idx(nbr, n_full_tiles=TF, tailp=TAILP):
    """Permute a [ns, 9] int32 neighbor-index shard into the gather layout.

    Returns [128, n_full_tiles*IDXC + (9 if tail else 0)] int32:
      per super-tile t, cols [0:64)  = I0[32*k'+j0, m] = nbr[2048t+32m+j0, k']
                    cols [64:128)    = same for k' in 4..8
                    cols [128:144)   = I2[32*s+j0, m2] = nbr[2048t+512s+32m2+j0, 8]
      tail block (vertices TF*2048 .. ns, zero-padded to 128):
        cols [0:4) = nbr[base+32m+j0, k'] k'=0..3; [4:8) k'=4..7; [8] = k=8.
    """
    ns = nbr.shape[0]
    nfull = n_full_tiles * VSUP
    a = np.asarray(nbr[:nfull], np.int32)
    A = a.reshape(n_full_tiles, 64, 32, K)             # [t, m, j0, k]
    i0 = A[..., 0:4].transpose(0, 3, 2, 1).reshape(n_full_tiles, 128, 64)
    i1 = A[..., 4:8].transpose(0, 3, 2, 1).reshape(n_full_tiles, 128, 64)
    # I2[32s+j0, m2] = nbr[2048t + 512s + 32m2 + j0, 8]
    i2 = (
        a[:, 8].reshape(n_full_tiles, 4, 16, 32)       # [t, s, m2, j0]
        .transpose(0, 1, 3, 2)                         # [t, s, j0, m2]
        .reshape(n_full_tiles, 128, 16)
    )
    idx = np.concatenate([i0, i1, i2], axis=2)         # [t, 128, 144]
    full = idx.transpose(1, 0, 2).reshape(128, n_full_tiles * IDXC)
    if not tailp:
        return np.ascontiguousarray(full)
    at = np.zeros((tailp, K), np.int32)
    at[:ns - nfull] = nbr[nfull:]
    At = at.reshape(4, 32, K)                          # [m, j0, k]
    t0 = At[..., 0:4].transpose(2, 1, 0).reshape(128, 4)
    t1 = At[..., 4:8].transpose(2, 1, 0).reshape(128, 4)
    t2 = at[:, 8].reshape(128, 1)
    return np.ascontiguousarray(np.concatenate([full, t0, t1, t2], axis=1))


def pack_weights(weight, bias_param):
    import ml_dtypes

    wp = np.zeros((128, 96), np.float32)
    wp[:, 0:32] = weight[0:128]
    wp[:, 32:64] = weight[128:256]
    wp[0:32, 64:96] = weight[256:288]
    bp = np.ascontiguousarray(np.tile(np.asarray(bias_param, np.float32), 4)[:, None])
    return wp.astype(ml_dtypes.bfloat16), bp


def make_in_maps(lattice_values, neighbor_indices, weight, bias_param):
    import ml_dtypes

    lat = np.ascontiguousarray(
        np.asarray(lattice_values, np.float32).astype(ml_dtypes.bfloat16)
    )
    nbr = np.asarray(neighbor_indices, np.int32)
    wp, bp = pack_weights(np.asarray(weight, np.float32), bias_param)
    return [
        {
            "lat": lat,
            "idx": prep_idx(nbr[c * NS:(c + 1) * NS]),
            "w": wp,
            "b": bp,
        }
        for c in range(NCORES)
    ]


def kernel(lattice_values, neighbor_indices, weight, bias_param):
    from concourse import bass_utils

    nc = get_nc()
    in_maps = make_in_maps(lattice_values, neighbor_indices, weight, bias_param)
    res = bass_utils.run_bass_kernel_spmd(nc, in_maps, core_ids=list(range(NCORES)))
    return np.ascontiguousarray(
        np.concatenate(
            [np.asarray(r["out"][:, :NS]).astype(np.float32).T for r in res.results],
            axis=0,
        )
    )
